# revision 29
# baseline (speedup 1.0000x reference)
"""CorrelationHead Trainium2 kernel (v2: parity-stacked fc1).

Math: SpatialCorrelationSampler(patch=16, dil=2) on 7x7 maps zero-pads x2 by
(14,16). corr[b] (12544 features) has exactly 2401 distinct nonzero values =
Gram matrix G[b][kl,ij] = sum_c x1[b,c,ij]*x2[b,c,kl], and only parity-valid
(kl,ij) pairs (k=i mod 2, l=j mod 2) ever appear in corr: 625 of 2401.
fc1(corr) = sum over the 625 valid pairs of G * W1eff, so we:
  1. gram:  per-RoI G^T[kl, ij] on PE (K=256 contraction, 2 matmuls),
     evict to SBUF gsa[49, 128b, 49ij] (DVE).
  2. stack: 49 selection matmuls (0/1 Sel, M=32-row slot, N=128b) scatter the
     625 valid (kl,ij) rows into 6 dense K<=128 "stacks" in PSUM, 2-3 ij per
     32-slot via PSUM accumulation. b1 folds in as a constant ones-row.
  3. fc1:   12 matmuls (6 stacks x 2 psum halves) with densely packed W1
     (1.57MB instead of 49x49x1024 = 4.9MB with 74% structural zeros).
  4. tail:  PE transpose + fc2 + transpose + fc3 (as v1).

DMA: x (6.4MB bf16) is the floor; 16 chunks alternate the two HWDGE queues
(SP/ACT) with 3.1KB per-partition contiguous runs; w2 halves trail x on the
same queues; Sel/W1/consts ride the gpsimd SWDGE queue.

Sharding: pure data-parallel over the 1024 RoIs -> 128 per each of 8 cores.
"""

import os
import numpy as np

import concourse.bass as bass
import concourse.mybir as mybir
from concourse.bass_utils import run_bass_kernel_spmd

# ---------------------------------------------------------------- constants
P = 16
DIL = 2
H = 7
C = 256
B = 1024
REP = 1024
HW = H * H  # 49
N_CORES = 8
BL = B // N_CORES  # 128 RoIs per core

NCHUNK = 16
CB = BL // NCHUNK        # 8 RoIs per chunk
NG = 4                   # RoIs per PSUM gram group
NGROUP = BL // NG        # 32
GPC = CB // NG           # groups per chunk = 2
CHW = 2 * 2 * CB * HW    # els per partition per chunk = 1568

F32 = mybir.dt.float32
BF16 = mybir.dt.bfloat16
FP8 = mybir.dt.float8e4

XDT = os.environ.get("CORR_XDT", "bf16")  # bf16 | fp8
WDELAY = int(os.environ.get("CORR_WDELAY", "4"))  # gp waits s_x[WDELAY-1] first
WARM = int(os.environ.get("CORR_WARM", "1"))  # HAM warmer dummy matmuls

LAST_EXEC_NS = None
_CACHE = {}


# ------------------------------------------------------------- stack layout
def _klist(ij):
    i, j = ij // H, ij % H
    return [k * H + l for k in range(i % 2, H, 2) for l in range(j % 2, H, 2)]


def _layout():
    """Slot/stack assignment for the 625 valid (kl,ij) pairs.

    Returns slots: list of lists of ij; slot t lives at stack t//4,
    partition base 32*(t%4). Each ij occupies rows [off, off+nkl) of its
    slot where off = sum of nkl of earlier ijs in the slot.
    """
    EE = [i * H + j for i in range(0, H, 2) for j in range(0, H, 2)]  # 16x16
    EO = [i * H + j for i in range(0, H, 2) for j in range(1, H, 2)]  # 12x12
    OE = [i * H + j for i in range(1, H, 2) for j in range(0, H, 2)]  # 12x12
    OO = [i * H + j for i in range(1, H, 2) for j in range(1, H, 2)]  # 9x9
    slots = []
    slots += [[EE[2 * t], EE[2 * t + 1]] for t in range(8)]           # 32 rows
    slots += [[EO[2 * t], EO[2 * t + 1]] for t in range(6)]           # 24 rows
    slots += [[OE[2 * t], OE[2 * t + 1]] for t in range(6)]           # 24 rows
    slots += [[OO[3 * t], OO[3 * t + 1], OO[3 * t + 2]] for t in range(3)]
    assert len(slots) == 23
    return slots


SLOTS = _layout()
NSTACK = 6
STACK_K = [128, 128, 128, 128, 128, 97]  # stack 5: 3 slots + bias row at 96


PHASE = os.environ.get("CORR_PHASE", "full")  # gram|perm|fc1|full


def _xdt():
    return FP8 if XDT == "fp8" else BF16


# ---------------------------------------------------------------- device IR
def _build():
    dt = BF16
    nc = bass.Bass()

    xdt = _xdt()
    xh = nc.dram_tensor("xh", [128, NCHUNK * CHW], xdt, kind="ExternalInput")
    selh = nc.dram_tensor("selh", [HW, HW * 32], dt, kind="ExternalInput")
    w1h = nc.dram_tensor("w1h", [128, NSTACK * REP], dt, kind="ExternalInput")
    w2h = nc.dram_tensor("w2h", [128, 8 * REP], dt, kind="ExternalInput")
    w3h = nc.dram_tensor("w3h", [128, 8 * 4], dt, kind="ExternalInput")
    b2h = nc.dram_tensor("b2h", [1, REP], dt, kind="ExternalInput")
    b3h = nc.dram_tensor("b3h", [1, 4], dt, kind="ExternalInput")
    onesh = nc.dram_tensor("onesh", [1, 128], dt, kind="ExternalInput")
    identh = nc.dram_tensor("identh", [128, 128], dt, kind="ExternalInput")
    zbh = nc.dram_tensor("zbh", [128, 1], F32, kind="ExternalInput")
    outh = nc.dram_tensor("outh", [128, 4], F32, kind="ExternalOutput")

    from contextlib import ExitStack

    with ExitStack() as ctx:
        sb = lambda name, shape, d: ctx.enter_context(nc.sbuf_tensor(name, shape, d))
        ps = lambda name, shape, d: ctx.enter_context(nc.psum_tensor(name, shape, d))
        sem = lambda name: ctx.enter_context(nc.semaphore(name))

        xs = sb("xs", [128, NCHUNK, CHW], xdt)
        gsa = sb("gsa", [HW, BL, HW], dt)
        sel = sb("sel", [HW, HW * 32], dt)
        w1s = sb("w1s", [128, NSTACK, REP], dt)
        w2s = sb("w2s", [128, 8 * REP], dt)
        w3s = sb("w3s", [128, 8 * 4], dt)
        Ssb = sb("Ssb", [128, NSTACK, 128], dt)
        b2s = sb("b2s", [1, REP], dt)
        b3s = sb("b3s", [1, 4], dt)
        ones = sb("ones", [1, 128], dt)
        idents = sb("idents", [128, 128], dt)
        zbias = sb("zbias", [128, 1], F32)
        relu1 = sb("relu1", [128, REP], dt)
        r1T = sb("r1T", [128, 8, 128], dt)
        relu2 = sb("relu2", [128, REP], dt)
        r2T = sb("r2T", [128, 8, 128], dt)
        outs = sb("outs", [128, 4], F32)

        psG0 = ps("psG0", [128, NG * HW], F32)
        psG1 = ps("psG1", [128, NG * HW], F32)
        psS0 = ps("psS0", [128, 4, 128], F32)
        psS1 = ps("psS1", [128, 260], F32)  # stacks 4,5 + psO in one bank
        psF0 = ps("psF0", [128, 512], F32)
        psF1 = ps("psF1", [128, 512], F32)
        psT0 = ps("psT0", [128, 128], dt)
        psT1 = ps("psT1", [128, 128], dt)
        psO = psS1[:, 256:260]

        s_x = [sem(f"s_x{i}") for i in range(NCHUNK)]
        s_w2a = sem("s_w2a")
        s_w2b = sem("s_w2b")
        s_sel = sem("s_sel")
        s_wA = sem("s_wA")
        s_wB = sem("s_wB")
        s_wC = sem("s_wC")
        s_wD = sem("s_wD")
        s_g = sem("s_g")
        s_ed = sem("s_ed")
        s_p0 = sem("s_p0")
        s_p1 = sem("s_p1")
        s_se = sem("s_se")
        s_se2 = sem("s_se2")
        s_f1 = sem("s_f1")
        s_r1 = sem("s_r1")
        s_t1 = sem("s_t1")
        s_c1 = sem("s_c1")
        s_ca1 = sem("s_ca1")
        s_f2 = sem("s_f2")
        s_r2 = sem("s_r2")
        s_t2 = sem("s_t2")
        s_c2 = sem("s_c2")
        s_ca2 = sem("s_ca2")
        s_f3 = sem("s_f3")
        s_oe = sem("s_oe")
        s_o = sem("s_o")

        block = ctx.enter_context(nc.Block())
        psG = [psG0, psG1]
        psF = [psF0, psF1]
        # 4-slot transpose ring: 2 dedicated bf16 banks + the (dead by then)
        # gram banks viewed as bf16
        psT = [
            psT0[:, :],
            psT1[:, :],
            psG0[:, 0:64].bitcast(BF16),
            psG1[:, 0:64].bitcast(BF16),
        ]

        # HAM warmer: a throwaway N=512 matmul into psF0 (clobbered by the
        # next start=True accumulation) to keep the PE clock-gate at 8/8
        def warm(pe, n=1, w=512):
            if not WARM:
                return
            for _ in range(n):
                pe.matmul(
                    psF0[:, 0:w], xs[:, 0, 0:128], xs[:, 0, 0:w],
                    start=True, stop=True,
                )

        # x slice helpers: chunk layout per partition = [t, h, b, ij]
        def xsl(ch, t, h, b):
            off = ((t * 2 + h) * CB + b) * HW
            return xs[:, ch, off : off + HW]

        # ---------------- SP: even x chunks, w2 half 0, final output
        @block.sync
        def _(sp):
            half = CHW // 2
            sp.dma_start(xs[:, 0, 0:half], xh[:, 0:half]).then_inc(s_x[0], 16)
            for ch in range(2, NCHUNK, 2):
                sp.dma_start(
                    xs[:, ch, :], xh[:, ch * CHW : (ch + 1) * CHW]
                ).then_inc(s_x[ch], 16)
            sp.wait_ge(s_x[NCHUNK - 1], 16)
            sp.dma_start(w2s[:, 0:4096], w2h[:, 0:4096]).then_inc(s_w2a, 16)
            sp.wait_ge(s_o, 16)

        # ---------------- GPSIMD: weights/constants via SWDGE
        @block.gpsimd
        def _(gp):
            gp.dma_start(sel[:, :], selh[:, :]).then_inc(s_sel, 16)
            gp.dma_start(ones[:, :], onesh[:, :]).then_inc(s_wA, 16)
            gp.dma_start(Ssb[96:97, 5, :], onesh[:, :]).then_inc(s_wA, 16)
            gp.dma_start(idents[:, :], identh[:, :]).then_inc(s_wB, 16)
            gp.dma_start(zbias[:, :], zbh[:, :]).then_inc(s_wC, 16)
            gp.dma_start(w3s[:, :], w3h[:, :]).then_inc(s_wD, 16)
            gp.dma_start(b2s[:, :], b2h[:, :]).then_inc(s_wD, 16)
            gp.dma_start(b3s[:, :], b3h[:, :]).then_inc(s_wD, 16)
            if WDELAY > 0:
                gp.wait_ge(s_x[WDELAY - 1], 32 if WDELAY == 1 else 16)
            gp.dma_start(w1s[:, :, :], w1h[:, :]).then_inc(s_wA, 16)

        # ---------------- ACT: odd x chunks, w2 half 1, psS evicts, relus
        @block.scalar
        def _(act):
            half = CHW // 2
            act.dma_start(xs[:, 0, half:CHW], xh[:, half:CHW]).then_inc(s_x[0], 16)
            for ch in range(1, NCHUNK, 2):
                act.dma_start(
                    xs[:, ch, :], xh[:, ch * CHW : (ch + 1) * CHW]
                ).then_inc(s_x[ch], 16)
            act.wait_ge(s_x[NCHUNK - 2], 16)
            act.dma_start(w2s[:, 4096:8192], w2h[:, 4096:8192]).then_inc(s_w2b, 16)

            if PHASE == "gram":
                act.wait_ge(s_ed, NGROUP)
                act.activation(
                    outs[0:49, :], gsa[:, 0, 0:4],
                    mybir.ActivationFunctionType.Copy,
                ).then_inc(s_oe, 1)
                return

            if PHASE == "perm":
                act.wait_ge(s_se, 2)
                act.wait_ge(s_se2, 2)
                act.activation(
                    outs[:, :], Ssb[:, 0, 0:4], mybir.ActivationFunctionType.Copy
                ).then_inc(s_oe, 1)
                return

            act.wait_ge(s_wC, 16)  # zbias
            for hf in range(2):
                act.wait_ge(s_f1, hf + 1)
                act.activation(
                    relu1[:, hf * 512 : (hf + 1) * 512], psF[hf][:, :],
                    mybir.ActivationFunctionType.Relu, bias=zbias[:, :],
                ).then_inc(s_r1, 1)
            if PHASE == "fc1":
                act.activation(
                    outs[:, :], relu1[:, 0:4], mybir.ActivationFunctionType.Copy
                ).then_inc(s_oe, 1)
                return

            for hf in range(2):
                act.wait_ge(s_f2, hf + 1)
                act.activation(
                    relu2[:, hf * 512 : (hf + 1) * 512], psF[hf][:, :],
                    mybir.ActivationFunctionType.Relu, bias=zbias[:, :],
                ).then_inc(s_r2, 1)
            act.wait_ge(s_f3, 1)
            act.activation(
                outs[:, :], psO, mybir.ActivationFunctionType.Copy
            ).then_inc(s_oe, 1)
            act.wait_ge(s_oe, 1)
            act.dma_start(outh[:, :], outs[:, :]).then_inc(s_o, 16)

        # ---------------- PE: all matmuls
        @block.tensor
        def _(pe):
            # stack: scatter valid (kl,ij) into dense stacks via 0/1 matmuls;
            # half 0 (b 0:64) interleaves under the half-1 gram DMA window
            def perm_half(hb):
                c0, c1 = 64 * hb, 64 * (hb + 1)
                for t, ijs in enumerate(SLOTS):
                    st, base = t // 4, 32 * (t % 4)
                    for u, ij in enumerate(ijs):
                        pst = (
                            psS0[base : base + 32, st, c0:c1]
                            if st < 4
                            else psS1[
                                base : base + 32, (st - 4) * 128 + c0 : (st - 4) * 128 + c1
                            ]
                        )
                        mm = pe.matmul(
                            pst,
                            sel[:, ij * 32 : (ij + 1) * 32],
                            gsa[:, c0:c1, ij],
                            start=(u == 0),
                            stop=(u == len(ijs) - 1),
                            tile_position=(0, base),
                        )
                    if hb == 1 and t == 15:
                        mm.then_inc(s_p0, 1)
                    if hb == 1 and t == 22:
                        mm.then_inc(s_p1, 1)
                if hb == 1 and t % 3 == 2:
                    warm(pe, 1, 256)

            # gram: G[b]^T[kl, ij] for each local RoI
            for ch in range(NCHUNK):
                pe.wait_ge(s_x[ch], 32 if ch == 0 else 16)
                for g in range(GPC):
                    gi = ch * GPC + g
                    q = gi % 2
                    if gi >= 2:
                        pe.wait_ge(s_ed, gi - 1)
                    for bb in range(NG):
                        lb = g * NG + bb
                        for h in range(2):
                            mm = pe.matmul(
                                psG[q][0:HW, bb * HW : (bb + 1) * HW],
                                xsl(ch, 1, h, lb),
                                xsl(ch, 0, h, lb),
                                start=(h == 0),
                                stop=(h == 1),
                            )
                    mm.then_inc(s_g, 1)
                if ch >= 10:
                    warm(pe, 2)
                if ch == NCHUNK // 2 and PHASE != "gram":
                    # half-0 RoIs all grammed; repack them while the second
                    # half of x is still streaming in
                    pe.wait_ge(s_ed, NGROUP // 2)
                    pe.wait_ge(s_sel, 16)
                    perm_half(0)

            if PHASE == "gram":
                return

            pe.wait_ge(s_ed, NGROUP)
            perm_half(1)

            if PHASE == "perm":
                return

            # fc1: psF[hf] += S[stack]^T @ W1[stack]
            pe.wait_ge(s_wA, 48)  # w1, ones, S bias row
            for hf in range(2):
                for s in range(NSTACK):
                    if hf == 0 and s == 0:
                        pe.wait_ge(s_se, 1)
                    if hf == 0 and s == 2:
                        pe.wait_ge(s_se, 2)
                    if hf == 0 and s == 4:
                        pe.wait_ge(s_se2, 1)
                    if hf == 0 and s == 5:
                        pe.wait_ge(s_se2, 2)
                    ks = STACK_K[s]
                    mm = pe.matmul(
                        psF[hf][:, :],
                        Ssb[0:ks, s, :],
                        w1s[0:ks, s, hf * 512 : hf * 512 + 512],
                        start=(s == 0),
                        stop=(s == NSTACK - 1),
                    )
                mm.then_inc(s_f1, 1)

            if PHASE == "fc1":
                return

            # transpose relu1 -> r1T (copybacks alternate DVE even / ACT odd)
            pe.wait_ge(s_wB, 16)  # idents
            for k in range(8):
                pe.wait_ge(s_r1, 1 if k < 4 else 2)
                if k >= 4:
                    pe.wait_ge(s_c1, k - 3)
                pe.transpose(
                    psT[k % 4], relu1[:, k * 128 : (k + 1) * 128], idents[:, :]
                ).then_inc(s_t1, 1)
                warm(pe, 1, 256)

            # fc2: bias first (off critical path), then 8 K-chunks with the
            # hf=1 matmul reusing the hf=0 stationary (ldweights=False)
            pe.wait_ge(s_w2a, 16)
            pe.wait_ge(s_w2b, 16)
            pe.wait_ge(s_wD, 48)  # w3, b2, b3
            pe.wait_ge(s_r1, 2)  # relu1 must have consumed psF
            for hf in range(2):
                pe.matmul(
                    psF[hf][:, :],
                    ones[:, :],
                    b2s[:, hf * 512 : hf * 512 + 512],
                    start=True,
                    stop=False,
                )
            for k in range(8):
                pe.wait_ge(s_c1, k + 1)
                for hf in range(2):
                    mm = pe.matmul(
                        psF[hf][:, :],
                        r1T[:, k, :],
                        w2s[:, k * REP + hf * 512 : k * REP + hf * 512 + 512],
                        start=False,
                        stop=(k == 7),
                    )
                    if k == 7:
                        mm.then_inc(s_f2, 1)

            # transpose relu2 -> r2T
            for k in range(8):
                pe.wait_ge(s_r2, 1 if k < 4 else 2)
                if k >= 4:
                    pe.wait_ge(s_c2, k - 3)
                pe.transpose(
                    psT[k % 4], relu2[:, k * 128 : (k + 1) * 128], idents[:, :]
                ).then_inc(s_t2, 1)
                if k < 7:
                    warm(pe, 1, 256)

            # fc3: bias first, then 8 K-chunks
            pe.matmul(psO, ones[:, :], b3s[:, :], start=True, stop=False)
            for k in range(8):
                pe.wait_ge(s_c2, k + 1)
                mm = pe.matmul(
                    psO,
                    r2T[:, k, :],
                    w3s[:, k * 4 : (k + 1) * 4],
                    start=False,
                    stop=(k == 7),
                )
            mm.then_inc(s_f3, 1)

        # ---------------- DVE: gram evictions + transpose copybacks
        @block.vector
        def _(dve):
            for gi in range(NGROUP):
                q = gi % 2
                dve.wait_ge(s_g, gi + 1)
                dve.tensor_copy(
                    gsa[:, gi * NG : (gi + 1) * NG, :], psG[q][0:HW, :]
                ).then_inc(s_ed, 1)
            if PHASE == "gram":
                return
            dve.wait_ge(s_p0, 1)
            dve.tensor_copy(Ssb[:, 0:2, :], psS0[:, 0:2, :]).then_inc(s_se, 1)
            dve.tensor_copy(Ssb[:, 2:4, :], psS0[:, 2:4, :]).then_inc(s_se, 1)
            dve.wait_ge(s_p1, 1)
            dve.tensor_copy(Ssb[:, 4, :], psS1[:, 0:128]).then_inc(s_se2, 1)
            dve.tensor_copy(Ssb[0:96, 5, :], psS1[0:96, 128:256]).then_inc(s_se2, 1)
            if PHASE != "full":
                return
            for k in range(8):
                dve.wait_ge(s_t1, k + 1)
                dve.tensor_copy(r1T[:, k, :], psT[k % 4]).then_inc(s_c1, 1)
            for k in range(8):
                dve.wait_ge(s_t2, k + 1)
                dve.tensor_copy(r2T[:, k, :], psT[k % 4]).then_inc(s_c2, 1)

    return nc


def _get_nc():
    key = ("nc", PHASE, XDT, WDELAY, WARM)
    if key not in _CACHE:
        _CACHE[key] = _build()
    return _CACHE[key]


# ---------------------------------------------------------------- host prep
def _prep_weights(W1, b1, np_dt):
    """W1 packed by stack layout + Sel matrices."""
    w1np = np.zeros((128, NSTACK, REP), dtype=np.float32)
    selnp = np.zeros((HW, HW * 32), dtype=np.float32)
    for t, ijs in enumerate(SLOTS):
        st, base = t // 4, 32 * (t % 4)
        off = 0
        for ij in ijs:
            i, j = ij // H, ij % H
            for m, kl in enumerate(_klist(ij)):
                k, l = kl // H, kl % H
                ph = (k - i) // 2 + 7
                pw = (l - j) // 2 + 7
                f = (ph * P + pw) * HW + ij
                w1np[base + off + m, st, :] = W1[:, f]
                selnp[kl, ij * 32 + off + m] = 1.0
            off += len(_klist(ij))
    w1np[96, 5, :] = b1
    return w1np.reshape(128, NSTACK * REP).astype(np_dt), selnp.astype(np_dt)


# ---------------------------------------------------------------- entry
def kernel(patch1, patch2, W1, b1, W2, b2, W3, b3):
    global LAST_EXEC_NS
    import ml_dtypes

    np_dt = ml_dtypes.bfloat16
    np_xdt = ml_dtypes.float8_e4m3 if XDT == "fp8" else ml_dtypes.bfloat16

    patch1 = np.asarray(patch1, dtype=np.float32).reshape(B, 2, 128, HW)
    patch2 = np.asarray(patch2, dtype=np.float32).reshape(B, 2, 128, HW)
    W1 = np.asarray(W1, dtype=np.float32)
    W2 = np.asarray(W2, dtype=np.float32)
    W3 = np.asarray(W3, dtype=np.float32)
    b1 = np.asarray(b1, dtype=np.float32)
    b2 = np.asarray(b2, dtype=np.float32)
    b3 = np.asarray(b3, dtype=np.float32)

    w1e, sele = _prep_weights(W1, b1, np_dt)
    w2e = np.ascontiguousarray(
        W2.T.reshape(8, 128, REP).transpose(1, 0, 2).reshape(128, 8 * REP)
    ).astype(np_dt)
    w3e = np.ascontiguousarray(
        W3.T.reshape(8, 128, 4).transpose(1, 0, 2).reshape(128, 32)
    ).astype(np_dt)

    shared = {
        "selh": sele,
        "w1h": w1e,
        "w2h": w2e,
        "w3h": w3e,
        "b2h": b2.reshape(1, REP).astype(np_dt),
        "b3h": b3.reshape(1, 4).astype(np_dt),
        "onesh": np.ones((1, 128), dtype=np_dt),
        "identh": np.eye(128, dtype=np.float32).astype(np_dt),
        "zbh": np.zeros((128, 1), dtype=np.float32),
    }

    in_maps = []
    for i in range(N_CORES):
        sl = slice(i * BL, (i + 1) * BL)
        # xh[c, ch, t, h, b, ij]
        xt = np.stack([patch1[sl], patch2[sl]], axis=0)  # [t, 128b, h, c, ij]
        xt = xt.reshape(2, NCHUNK, CB, 2, 128, HW).transpose(4, 1, 0, 3, 2, 5)
        xh = np.ascontiguousarray(xt).reshape(128, NCHUNK * CHW).astype(np_xdt)
        in_maps.append({"xh": xh, **shared})

    nc = _get_nc()
    trace = os.environ.get("CORR_TRACE", "0") == "1"
    res = run_bass_kernel_spmd(nc, in_maps, list(range(N_CORES)), trace=trace)
    LAST_EXEC_NS = res.exec_time_ns

    out = np.concatenate(
        [res.results[i]["outh"] for i in range(N_CORES)], axis=0
    ).astype(np.float32)
    return out
